# revision 12
# baseline (speedup 1.0000x reference)
"""GATv2 layer (N=50000, D=128, H=4, E=600000) on 8 trn2 NeuronCores.

Layout: one destination node per SBUF partition row. Nodes are globally
sorted by in-degree, striped across the 8 cores (rank % 8), and packed
into 49 windows of 128 nodes per core; window w pads every node's edge
list to S[w] slots (S[w] = max degree in that window across cores), so
high-degree nodes share windows and padding stays small.

Per window: two dma_gather instructions pull xl[src] rows (bf16, 256B
elements) for all 128*S[w] edge slots — the xl table is split at row
32768 because dma_gather indices are signed int16; slots whose row
lives in the other half gather a dedicated all-zero row, so the halves
just add (no select). xr[dst] is the node's own row broadcast along the
slot axis (no second gather, no selection matmuls). Segment "softmax"
and the alpha-weighted aggregation are row-local vector reduces.

NOTE: this environment's jax.ops.segment_max computes a segment SUM;
the reference subtracts that (not the max) before exp and divides by
(den + 1e-16). We reproduce both quirks exactly — they change the
output materially (some heads collapse to ~0 when den << 1e-16).

xl table rows are stored chunk-permuted (pi) so the phase-1 table write
is one contiguous 4KB-per-partition DMA per 16-tile chunk; the host
remaps gather indices accordingly and un-permutes the output.
"""

import math
import numpy as np
import ml_dtypes

import concourse.bass as bass
import concourse.bacc as bacc
import concourse.mybir as mybir
import concourse.tile as tile
from concourse.masks import make_identity
from concourse.bass_utils import run_bass_kernel_spmd

P = 128
F32 = mybir.dt.float32
BF16 = mybir.dt.bfloat16
I32 = mybir.dt.int32
I16 = mybir.dt.int16
BFNP = ml_dtypes.bfloat16

NEG_SLOPE = 0.2
BN_EPS = 1e-5
HALF = 32768          # dma_gather signed-int16 index limit


class Cfg:
    def __init__(self, N, D, H, n_cores, S):
        assert D == P
        self.N, self.D, self.H = N, D, H
        self.C = D // H
        self.n_cores = n_cores
        self.NPC = N // n_cores              # nodes per core
        self.W = math.ceil(self.NPC / P)     # windows per core
        self.NROWS = self.W * P
        self.LASTR = self.NPC - P * (self.W - 1)
        self.NT = math.ceil(N / P)           # xl table tiles
        self.TAB = self.NT * P
        # [zeros 128] [table TAB] [zeros 128]
        self.TABX = self.TAB + 2 * P
        self.BZERO = self.TAB + P - HALF     # back zero row, rel to HALF
        self.CH = 16                         # tiles per xl-table write chunk
        self.NCH = math.ceil(self.NT / self.CH)
        self.S = [int(s) for s in S]         # slots per window
        offs = np.concatenate([[0], np.cumsum(self.S)])
        self.offs = [int(o) for o in offs]
        self.SS = int(offs[-1])
        self.Smax = int(max(self.S))
        self.J = 7                           # windows per output write group
        self.NG = self.W // self.J
        assert self.W == self.J * self.NG


def build_kernel(nc: bass.Bass, cfg: Cfg, no_gather=False, single_q=False):
    N, H, C, W = cfg.N, cfg.H, cfg.C, cfg.W
    NT, TAB, TABX, CH, NCH = cfg.NT, cfg.TAB, cfg.TABX, cfg.CH, cfg.NCH
    SS, Smax, LASTR = cfg.SS, cfg.Smax, cfg.LASTR
    J, NG = cfg.J, cfg.NG

    # ---- I/O ----
    hfullT = nc.declare_dram_parameter("hfullT", [P, TAB], BF16, isOutput=False)
    hlocT = nc.declare_dram_parameter("hlocT", [P, cfg.NROWS], BF16,
                                      isOutput=False)
    hres2 = nc.declare_dram_parameter("hres2", [cfg.NROWS, P], F32,
                                      isOutput=False)
    constsW = nc.declare_dram_parameter("constsW", [P, 2 * P], BF16,
                                        isOutput=False)
    constsF = nc.declare_dram_parameter("constsF", [P, 5], F32, isOutput=False)
    idxa = nc.declare_dram_parameter("idxa", [P, SS * 8], I16, isOutput=False)
    idxb = nc.declare_dram_parameter("idxb", [P, SS * 8], I16, isOutput=False)
    maskb = nc.declare_dram_parameter("maskb", [P, SS], F32, isOutput=False)
    out = nc.declare_dram_parameter("out", [cfg.NROWS, P], F32, isOutput=True)

    # ---- internal DRAM ----
    xl_tab = nc.dram_tensor("xl_tab", [TABX, P], BF16)
    st_in = nc.dram_tensor("st_in", [P, 2], F32)
    st_out = nc.dram_tensor("st_out", [P, 2], F32, addr_space="Shared")

    with tile.TileContext(nc) as tc:
        import contextlib
        with contextlib.ExitStack() as ctx:
            cst = ctx.enter_context(tc.tile_pool(name="cst", bufs=1))
            ps = ctx.enter_context(tc.tile_pool(name="ps", bufs=4, space="PSUM"))
            ps1 = ctx.enter_context(tc.tile_pool(name="ps1", bufs=1,
                                                 space="PSUM"))

            # ================= constants =================
            csWl = cst.tile([P, P], BF16, tag="csWl")
            nc.sync.dma_start(out=csWl[:], in_=constsW[:, 0:P])
            csWr = cst.tile([P, P], BF16, tag="csWr")
            nc.sync.dma_start(out=csWr[:], in_=constsW[:, P:2 * P])
            csF = cst.tile([P, 5], F32, tag="csF")
            nc.sync.dma_start(out=csF[:], in_=constsF[:])
            ia_sb = cst.tile([P, SS * 8], I16, tag="ia_sb")
            nc.sync.dma_start(out=ia_sb[:], in_=idxa[:])
            ib_sb = cst.tile([P, SS * 8], I16, tag="ib_sb")
            nc.sync.dma_start(out=ib_sb[:], in_=idxb[:])
            msk_sb = cst.tile([P, SS], F32, tag="msk_sb")
            nc.sync.dma_start(out=msk_sb[:], in_=maskb[:])
            msk16 = cst.tile([P, SS], BF16, tag="msk16")
            nc.scalar.copy(msk16[:], msk_sb[:])

            att_col = csF[:, 0:1]
            gam_col = csF[:, 1:2]
            bet_col = csF[:, 2:3]

            ones_col = cst.tile([P, 1], F32, tag="ones_c")
            nc.gpsimd.memset(ones_col[:], 1.0)
            ident = cst.tile([P, P], F32, tag="ident")
            make_identity(nc, ident[:])
            eps_col = cst.tile([P, 1], F32, tag="epsc")
            nc.gpsimd.memset(eps_col[:], BN_EPS)
            ones_msk = cst.tile([P, 1], F32, tag="ones_m")
            if LASTR < P:
                pidx = cst.tile([P, 1], I32, tag="pidx")
                nc.gpsimd.iota(pidx[:], pattern=[[0, 1]], channel_multiplier=1)
                nc.vector.tensor_scalar(out=ones_msk[:], in0=pidx[:],
                                        scalar1=LASTR, scalar2=None,
                                        op0=mybir.AluOpType.is_lt)
            else:
                nc.gpsimd.memset(ones_msk[:], 1.0)

            # att replicated to all partitions (bf16 row), pre-scaled 0.6
            att_ps = ps.tile([P, P], F32, tag="p1")
            nc.tensor.transpose(att_ps[:], att_col.to_broadcast([P, P]),
                                ident[:])
            att16 = cst.tile([P, P], BF16, tag="att16")
            nc.scalar.activation(att16[:], att_ps[:],
                                 mybir.ActivationFunctionType.Copy,
                                 scale=(1.0 + NEG_SLOPE) / 2.0)
            # bias_l / bias_r replicated to all partitions (f32 rows)
            bl_ps = ps.tile([P, P], F32, tag="p1")
            nc.tensor.transpose(bl_ps[:], csF[:, 3:4].to_broadcast([P, P]),
                                ident[:])
            bl_rep = cst.tile([P, P], F32, tag="blrep")
            nc.scalar.copy(bl_rep[:], bl_ps[:])
            br_ps = ps.tile([P, P], F32, tag="p1")
            nc.tensor.transpose(br_ps[:], csF[:, 4:5].to_broadcast([P, P]),
                                ident[:])
            br_rep = cst.tile([P, P], F32, tag="brrep")
            nc.scalar.copy(br_rep[:], br_ps[:])

            # resident per-window data
            xr16 = cst.tile([P, cfg.NROWS], BF16, tag="xr16")
            xr32 = cst.tile([P, cfg.NROWS], F32, tag="xr32")
            outpre = []
            for w in range(W):
                op_w = cst.tile([P, P], F32, tag=f"op{w}")
                outpre.append(op_w)

            # ================= phase 1: xl table (all nodes) =================
            with tc.tile_pool(name="sb1", bufs=3) as sb1:
                # dedicated all-zero rows at both ends of the table
                ztile = sb1.tile([P, P], BF16, tag="ztile")
                nc.gpsimd.memset(ztile[:], 0.0)
                nc.sync.dma_start(
                    out=xl_tab[0:P, :].rearrange("(p x) f -> p (x f)", p=P),
                    in_=ztile[:])
                nc.sync.dma_start(
                    out=xl_tab[TAB + P:TABX, :]
                        .rearrange("(p x) f -> p (x f)", p=P),
                    in_=ztile[:])

                for c in range(NCH):
                    wd = min(CH, NT - c * CH)
                    c0 = c * CH * P
                    hc = sb1.tile([P, CH * P], BF16, tag="hc")
                    nc.sync.dma_start(out=hc[:, :wd * P],
                                      in_=hfullT[:, c0:c0 + wd * P])
                    xlc = sb1.tile([P, CH * P], BF16, tag="xlc")
                    for j in range(wd):
                        p1 = ps.tile([P, P], F32, tag="p1")
                        nc.tensor.matmul(p1[:], lhsT=hc[:, j * P:(j + 1) * P],
                                         rhs=csWl[:],
                                         start=True, stop=True)
                        nc.vector.tensor_add(xlc[:, j * P:(j + 1) * P],
                                             p1[:], bl_rep[:])
                    # rows at P+c0 stored partition-major: row = P+c0+p*wd+j
                    nc.sync.dma_start(
                        out=xl_tab[P + c0:P + c0 + wd * P, :]
                            .rearrange("(p x) f -> p (x f)", p=P),
                        in_=xlc[:, :wd * P])

                # ---- phase 1b: xr for local (permuted) nodes ----
                hl = sb1.tile([P, cfg.NROWS], BF16, tag="hl")
                nc.sync.dma_start(out=hl[:], in_=hlocT[:])
                for w in range(W):
                    p1 = ps.tile([P, P], F32, tag="p1")
                    nc.tensor.matmul(p1[:], lhsT=hl[:, w * P:(w + 1) * P],
                                     rhs=csWr[:],
                                     start=True, stop=True)
                    nc.vector.tensor_add(xr16[:, w * P:(w + 1) * P],
                                         p1[:], br_rep[:])
                    # upcast of the bf16 value => exact cancellation later
                    nc.vector.tensor_copy(xr32[:, w * P:(w + 1) * P],
                                          xr16[:, w * P:(w + 1) * P])

            tc.strict_bb_all_engine_barrier()

            # ================= phase 2: per-window edge processing ==========
            stats_ps = ps1.tile([P, 2], F32, tag="stats")
            with tc.tile_pool(name="sb2", bufs=2) as sb2:
                for w in range(W):
                    S = cfg.S[w]
                    off = cfg.offs[w]
                    NI = S * P
                    wsl = slice(w * P, (w + 1) * P)

                    GA = sb2.tile([P, Smax, P], BF16, tag="GA")
                    GB = sb2.tile([P, Smax, P], BF16, tag="GB")
                    if no_gather:
                        nc.gpsimd.memset(GA[:, :S, :], 0.01)
                        nc.gpsimd.memset(GB[:, :S, :], 0.01)
                    else:
                        nc.gpsimd.dma_gather(
                            out_ap=GA[:, :S, :], in_ap=xl_tab[:HALF, :],
                            idxs_ap=ia_sb[:, off * 8:(off + S) * 8],
                            num_idxs=NI, num_idxs_reg=NI, elem_size=P,
                            queue_num=0, single_packet=False)
                        nc.gpsimd.dma_gather(
                            out_ap=GB[:, :S, :], in_ap=xl_tab[HALF:, :],
                            idxs_ap=ib_sb[:, off * 8:(off + S) * 8],
                            num_idxs=NI, num_idxs_reg=NI, elem_size=P,
                            queue_num=0 if single_q else 1,
                            single_packet=False)

                    # y = xl[src] + xr[dst]  (dst == own row)
                    Y = sb2.tile([P, Smax, P], BF16, tag="Y")
                    nc.vector.tensor_add(Y[:, :S, :], GA[:, :S, :],
                                         GB[:, :S, :])
                    nc.vector.tensor_add(
                        Y[:, :S, :], Y[:, :S, :],
                        xr16[:, wsl][:, None, :].to_broadcast([P, S, P]))

                    # z = LeakyReLU(y) = 0.6*y + 0.4*|y|; zz = z*att (inplace)
                    AB = sb2.tile([P, Smax, P], BF16, tag="AB")
                    nc.scalar.activation(
                        AB[:, :S, :], Y[:, :S, :],
                        mybir.ActivationFunctionType.Abs,
                        scale=(1.0 - NEG_SLOPE) / (1.0 + NEG_SLOPE))
                    nc.vector.tensor_add(AB[:, :S, :], AB[:, :S, :],
                                         Y[:, :S, :])
                    nc.vector.tensor_mul(
                        AB[:, :S, :], AB[:, :S, :],
                        att16[:, None, :].to_broadcast([P, S, P]))

                    # scores [p, h, s] = sum_c zz
                    s16 = sb2.tile([P, H, Smax], F32, tag="s16")
                    nc.vector.tensor_reduce(
                        out=s16[:, :, :S].rearrange("p h s -> p s h")
                            [:, :, :, None],
                        in_=AB[:, :S, :].rearrange("p s (h c) -> p s h c",
                                                   c=C),
                        op=mybir.AluOpType.add, axis=mybir.AxisListType.X)
                    # zero pad slots (multiplicative mask)
                    sm = sb2.tile([P, H, Smax], F32, tag="sm")
                    nc.vector.tensor_mul(
                        sm[:, :, :S], s16[:, :, :S],
                        msk_sb[:, off:off + S][:, None, :]
                            .to_broadcast([P, H, S]))
                    # segment-SUM subtraction (reference quirk), exp
                    m = sb2.tile([P, H], F32, tag="m")
                    nc.vector.tensor_reduce(
                        out=m[:, :, None], in_=sm[:, :, :S],
                        op=mybir.AluOpType.add, axis=mybir.AxisListType.X)
                    d = sb2.tile([P, H, Smax], F32, tag="d")
                    nc.vector.tensor_sub(
                        d[:, :, :S], sm[:, :, :S],
                        m[:, :, None].to_broadcast([P, H, S]))
                    e16 = sb2.tile([P, H, Smax], BF16, tag="e16")
                    nc.scalar.activation(e16[:, :, :S], d[:, :, :S],
                                         mybir.ActivationFunctionType.Exp)
                    em = sb2.tile([P, H, Smax], BF16, tag="em")
                    nc.vector.tensor_mul(
                        em[:, :, :S], e16[:, :, :S],
                        msk16[:, off:off + S][:, None, :]
                            .to_broadcast([P, H, S]))
                    den = sb2.tile([P, H], F32, tag="den")
                    nc.vector.tensor_reduce(
                        out=den[:, :, None], in_=em[:, :, :S],
                        op=mybir.AluOpType.add, axis=mybir.AxisListType.X)
                    den2 = sb2.tile([P, H], F32, tag="den2")
                    nc.vector.tensor_scalar(out=den2[:], in0=den[:],
                                            scalar1=1e-16, scalar2=None,
                                            op0=mybir.AluOpType.add)
                    rec = sb2.tile([P, H], F32, tag="rec")
                    nc.vector.reciprocal(rec[:], den2[:])
                    fden = sb2.tile([P, H], F32, tag="fden")
                    nc.vector.tensor_mul(fden[:], den[:], rec[:])

                    # weighted aggregation of y, then /(den+eps) and -xr*f
                    WM = sb2.tile([P, Smax, P], BF16, tag="WM")
                    nc.vector.tensor_mul(
                        WM[:, :S, :].rearrange("p s (h c) -> p s h c", c=C),
                        Y[:, :S, :].rearrange("p s (h c) -> p s h c", c=C),
                        em[:, :, :S].rearrange("p h s -> p s h")
                            [:, :, :, None].to_broadcast([P, S, H, C]))
                    op_w = outpre[w]
                    nc.vector.tensor_reduce(
                        out=op_w[:].rearrange("p (h c) -> p h c", c=C)
                            [:, :, :, None],
                        in_=WM[:, :S, :].rearrange("p s (h c) -> p h c s",
                                                   c=C),
                        op=mybir.AluOpType.add, axis=mybir.AxisListType.X)
                    nc.vector.tensor_mul(
                        op_w[:].rearrange("p (h c) -> p h c", c=C),
                        op_w[:].rearrange("p (h c) -> p h c", c=C),
                        rec[:, :, None].to_broadcast([P, H, C]))
                    xrf = sb2.tile([P, P], F32, tag="xrf")
                    nc.vector.tensor_mul(
                        xrf[:].rearrange("p (h c) -> p h c", c=C),
                        xr32[:, wsl].rearrange("p (h c) -> p h c", c=C),
                        fden[:, :, None].to_broadcast([P, H, C]))
                    nc.vector.tensor_sub(op_w[:], op_w[:], xrf[:])

                    # BN stats accumulation
                    sq = sb2.tile([P, P], F32, tag="sq")
                    nc.vector.tensor_mul(sq[:], op_w[:], op_w[:])
                    stat_ones = ones_msk if w == W - 1 else ones_col
                    nc.tensor.matmul(stats_ps[:, 0:1], lhsT=op_w[:],
                                     rhs=stat_ones[:],
                                     start=(w == 0), stop=(w == W - 1))
                    nc.tensor.matmul(stats_ps[:, 1:2], lhsT=sq[:],
                                     rhs=stat_ones[:],
                                     start=(w == 0), stop=(w == W - 1))

            # ================= phase 3: BN stats AllReduce =================
            with tc.tile_pool(name="sb3", bufs=2) as sb:
                st_sb = sb.tile([P, 2], F32, tag="stsb")
                nc.scalar.copy(st_sb[:], stats_ps[:])
                nc.sync.dma_start(out=st_in[:], in_=st_sb[:])
                tc.strict_bb_all_engine_barrier()
                nc.gpsimd.collective_compute(
                    "AllReduce", mybir.AluOpType.add,
                    replica_groups=[list(range(cfg.n_cores))],
                    ins=[st_in[:]], outs=[st_out[:]])
                tc.strict_bb_all_engine_barrier()
                st_all = sb.tile([P, 2], F32, tag="stall")
                nc.sync.dma_start(out=st_all[:], in_=st_out[:])

                # A = gamma * rsqrt(var+eps); B = beta - A*mu  (y = A*x + B)
                mu_c = sb.tile([P, 1], F32, tag="mu")
                nc.scalar.mul(mu_c[:], st_all[:, 0:1], 1.0 / N)
                ex2 = sb.tile([P, 1], F32, tag="ex2")
                nc.scalar.mul(ex2[:], st_all[:, 1:2], 1.0 / N)
                mu2 = sb.tile([P, 1], F32, tag="mu2")
                nc.scalar.square(mu2[:], mu_c[:])
                var_c = sb.tile([P, 1], F32, tag="var")
                nc.vector.tensor_sub(var_c[:], ex2[:], mu2[:])
                sd = sb.tile([P, 1], F32, tag="sd")
                nc.scalar.activation(sd[:], var_c[:],
                                     mybir.ActivationFunctionType.Sqrt,
                                     bias=eps_col[:])
                rsd = sb.tile([P, 1], F32, tag="rsd")
                nc.vector.reciprocal(rsd[:], sd[:])
                A_c = sb.tile([P, 1], F32, tag="Ac")
                nc.vector.tensor_mul(A_c[:], gam_col, rsd[:])
                Amu = sb.tile([P, 1], F32, tag="Amu")
                nc.vector.tensor_mul(Amu[:], A_c[:], mu_c[:])
                B_c = sb.tile([P, 1], F32, tag="Bc")
                nc.vector.tensor_sub(B_c[:], bet_col, Amu[:])

                A_ps = ps.tile([P, P], F32, tag="p1")
                nc.tensor.transpose(A_ps[:], A_c[:].to_broadcast([P, P]),
                                    ident[:])
                A_rep = cst.tile([P, P], F32, tag="Arep")
                nc.scalar.copy(A_rep[:], A_ps[:])
                B_ps = ps.tile([P, P], F32, tag="p1")
                nc.tensor.transpose(B_ps[:], B_c[:].to_broadcast([P, P]),
                                    ident[:])
                B_rep = cst.tile([P, P], F32, tag="Brep")
                nc.scalar.copy(B_rep[:], B_ps[:])

                # ============ phase 4: BN apply + relu + residual ==========
                for g in range(NG):
                    hres = sb.tile([P, J, P], F32, tag="hres")
                    nc.sync.dma_start(
                        out=hres[:],
                        in_=hres2[g * J * P:(g + 1) * J * P, :]
                            .rearrange("(p j) f -> p j f", p=P))
                    obuf = sb.tile([P, J, P], F32, tag="obuf")
                    for j in range(J):
                        w = g * J + j
                        t1 = sb.tile([P, P], F32, tag="t1")
                        nc.vector.tensor_mul(t1[:], outpre[w][:], A_rep[:])
                        t2 = sb.tile([P, P], F32, tag="t2")
                        nc.vector.tensor_add(t2[:], t1[:], B_rep[:])
                        r = sb.tile([P, P], F32, tag="r")
                        nc.scalar.activation(
                            r[:], t2[:], mybir.ActivationFunctionType.Relu)
                        nc.vector.tensor_add(obuf[:, j, :], r[:],
                                             hres[:, j, :])
                    nc.sync.dma_start(
                        out=out[g * J * P:(g + 1) * J * P, :]
                            .rearrange("(p j) f -> p j f", p=P),
                        in_=obuf[:])
    return nc


def host_prepare(h, edge_index, W_l, W_r, bias_l, bias_r, att,
                 bias_out, gamma, beta, n_cores=8):
    N, D = h.shape
    H, C = att.shape
    h = np.asarray(h, np.float32)
    ei = np.asarray(edge_index)

    loops = np.arange(N, dtype=np.int64)
    src = np.concatenate([ei[0], loops]).astype(np.int64)
    dst = np.concatenate([ei[1], loops]).astype(np.int64)
    deg = np.bincount(dst, minlength=N).astype(np.int64)

    # global degree-desc ordering, striped over cores: rank r -> core r%8
    order = np.argsort(-deg, kind="stable")
    NPC = N // n_cores
    node_of = order.reshape(NPC, n_cores)        # [pos, core]
    degmat = deg[node_of]                        # [pos, core]

    W = math.ceil(NPC / P)
    S = np.zeros(W, np.int64)
    for w in range(W):
        i0, i1 = w * P, min((w + 1) * P, NPC)
        S[w] = max(1, degmat[i0:i1].max())
    cfg = Cfg(N=N, D=D, H=H, n_cores=n_cores, S=S)

    # xl table row permutation (chunked partition-major storage)
    n = np.arange(cfg.TAB, dtype=np.int64)
    cc = n // (cfg.CH * P)
    jj = (n % (cfg.CH * P)) // P
    pp = n % P
    width = np.minimum(cfg.CH, cfg.NT - cc * cfg.CH)
    pi = cc * cfg.CH * P + pp * width + jj
    # node n -> table row P + pi[n]
    rowof = P + pi

    # edges grouped by dst, sorted by table row within each group
    eorder = np.lexsort((rowof[src], dst))
    row_s = rowof[src[eorder]].astype(np.int64)
    starts = np.zeros(N + 1, np.int64)
    starts[1:] = np.cumsum(deg)

    # shared inputs
    hT = np.zeros((P, cfg.TAB), BFNP)
    hT[:, :N] = h.T.astype(BFNP)
    constsW = np.zeros((P, 2 * P), BFNP)
    constsW[:, 0:P] = np.asarray(W_l, np.float32).astype(BFNP)
    constsW[:, P:2 * P] = np.asarray(W_r, np.float32).astype(BFNP)
    constsF = np.zeros((P, 5), np.float32)
    constsF[:, 0] = np.asarray(att, np.float32).reshape(-1)
    constsF[:, 1] = np.asarray(gamma, np.float32)
    constsF[:, 2] = np.asarray(beta, np.float32)
    constsF[:, 3] = np.asarray(bias_l, np.float32)
    constsF[:, 4] = np.asarray(bias_r, np.float32)

    # output/hres row packing: row2(w, p) = (w//J)*J*P + p*J + (w%J)
    wq, wr = np.divmod(np.arange(W), cfg.J)
    r2map = (wq[:, None] * (cfg.J * P) + np.arange(P)[None, :] * cfg.J
             + wr[:, None])                      # [W, P]

    offs = np.asarray(cfg.offs)
    pos = np.arange(NPC)
    w_k = pos // P
    p_k = pos % P
    colstart = offs[w_k]

    in_maps = []
    for k in range(n_cores):
        nodes = node_of[:, k]
        d_k = deg[nodes]
        tot = int(d_k.sum())
        cum = np.zeros(NPC + 1, np.int64)
        cum[1:] = np.cumsum(d_k)
        within = np.arange(tot, dtype=np.int64) - np.repeat(cum[:-1], d_k)
        rows = np.repeat(p_k, d_k)
        cols = np.repeat(colstart, d_k) + within
        eidx = np.repeat(starts[nodes], d_k) + within

        # slot table row values; pads use the dedicated zero rows
        vals = np.full((P, cfg.SS), -1, np.int64)
        vals[rows, cols] = row_s[eidx]
        isb = vals >= HALF
        ispad = vals < 0
        va = np.where(isb | ispad, 0, vals)              # front zero row
        vb = np.where(isb, vals - HALF, cfg.BZERO)       # back zero row
        maskba = np.zeros((P, cfg.SS), np.float32)
        maskba[rows, cols] = 1.0

        # dma_gather index layout: flat i = s*128+p; wrapped in 16
        # partitions, replicated down all 128 partitions
        def mk_idx(vmat):
            blocks = []
            for w in range(cfg.W):
                o, s_w = cfg.offs[w], cfg.S[w]
                flat = vmat[:, o:o + s_w].T.reshape(-1)   # [s*128+p]
                blk = flat.reshape(s_w * 8, 16).T         # [16, s*8]
                blocks.append(np.tile(blk, (8, 1)))
            return np.concatenate(blocks, axis=1).astype(np.int16)

        idxa = mk_idx(va)
        idxb = mk_idx(vb)

        hlocT = np.zeros((P, cfg.NROWS), BFNP)
        hlocT[:, :NPC] = h[nodes].T.astype(BFNP)
        hres2 = np.zeros((cfg.NROWS, P), np.float32)
        hres2[r2map[w_k, p_k]] = h[nodes]

        in_maps.append({
            "hfullT": hT, "hlocT": hlocT, "hres2": hres2,
            "constsW": constsW, "constsF": constsF,
            "idxa": idxa, "idxb": idxb, "maskb": maskba,
        })

    meta = {"node_of": node_of, "r2map": r2map, "w_k": w_k, "p_k": p_k}
    return cfg, in_maps, meta


def assemble_output(results, cfg, meta, n_cores=8):
    N = cfg.N
    rowsel = meta["r2map"][meta["w_k"], meta["p_k"]]   # [NPC]
    out_full = np.zeros((N, cfg.D), np.float32)
    for k in range(n_cores):
        vals = np.asarray(results[k]["out"], np.float32)[rowsel]
        out_full[meta["node_of"][:, k]] = vals
    return out_full


def kernel(h, edge_index, W_l, W_r, bias_l, bias_r, att,
           bias_out, gamma, beta):
    n_cores = 8
    cfg, in_maps, meta = host_prepare(h, edge_index, W_l, W_r, bias_l,
                                      bias_r, att, bias_out, gamma, beta,
                                      n_cores=n_cores)
    nc = bacc.Bacc(num_swdge_queues=2)
    build_kernel(nc, cfg)
    nc.compile()
    res = run_bass_kernel_spmd(nc, in_maps, core_ids=list(range(n_cores)))
    return assemble_output(res.results, cfg, meta, n_cores=n_cores)


# revision 14
# speedup vs baseline: 1.0484x; 1.0484x over previous
"""GATv2 layer (N=50000, D=128, H=4, E=600000) on 8 trn2 NeuronCores.

Layout: one destination node per SBUF partition row. Nodes are globally
sorted by in-degree, striped across the 8 cores (rank % 8), and packed
into 49 windows of 128 nodes per core; window w pads every node's edge
list to S[w] slots (S[w] = max degree in that window across cores), so
high-degree nodes share windows and padding stays small.

Per window: two dma_gather instructions pull xl[src] rows (bf16, 256B
elements) for all 128*S[w] edge slots — the xl table is split at row
32768 because dma_gather indices are signed int16; slots whose row
lives in the other half gather a dedicated all-zero row, so the halves
just add (no select). xr[dst] is the node's own row broadcast along the
slot axis (no second gather, no selection matmuls). Segment "softmax"
and the alpha-weighted aggregation are row-local vector reduces.

NOTE: this environment's jax.ops.segment_max computes a segment SUM;
the reference subtracts that (not the max) before exp and divides by
(den + 1e-16). We reproduce both quirks exactly — they change the
output materially (some heads collapse to ~0 when den << 1e-16).

xl table rows are stored chunk-permuted (pi) so the phase-1 table write
is one contiguous 4KB-per-partition DMA per 16-tile chunk; the host
remaps gather indices accordingly and un-permutes the output.
"""

import math
import numpy as np
import ml_dtypes

import concourse.bass as bass
import concourse.bacc as bacc
import concourse.mybir as mybir
import concourse.tile as tile
from concourse.masks import make_identity
from concourse.bass_utils import run_bass_kernel_spmd

P = 128
F32 = mybir.dt.float32
BF16 = mybir.dt.bfloat16
I32 = mybir.dt.int32
I16 = mybir.dt.int16
I8 = mybir.dt.int8
BFNP = ml_dtypes.bfloat16

NEG_SLOPE = 0.2
BN_EPS = 1e-5
HALF = 32768          # dma_gather signed-int16 index limit


class Cfg:
    def __init__(self, N, D, H, n_cores, S):
        assert D == P
        self.N, self.D, self.H = N, D, H
        self.C = D // H
        self.n_cores = n_cores
        self.NPC = N // n_cores              # nodes per core
        self.W = math.ceil(self.NPC / P)     # windows per core
        self.NROWS = self.W * P
        self.LASTR = self.NPC - P * (self.W - 1)
        self.NT = math.ceil(N / P)           # xl table tiles
        self.TAB = self.NT * P
        # [zeros 128] [table TAB] [zeros 128]
        self.TABX = self.TAB + 2 * P
        self.BZERO = self.TAB + P - HALF     # back zero row, rel to HALF
        self.CH = 16                         # tiles per xl-table write chunk
        self.NCH = math.ceil(self.NT / self.CH)
        self.S = [int(s) for s in S]         # slots per window
        offs = np.concatenate([[0], np.cumsum(self.S)])
        self.offs = [int(o) for o in offs]
        self.SS = int(offs[-1])
        self.Smax = int(max(self.S))
        self.J = 7                           # windows per output write group
        self.NG = self.W // self.J
        assert self.W == self.J * self.NG


def build_kernel(nc: bass.Bass, cfg: Cfg, no_gather=False, single_q=False):
    N, H, C, W = cfg.N, cfg.H, cfg.C, cfg.W
    NT, TAB, TABX, CH, NCH = cfg.NT, cfg.TAB, cfg.TABX, cfg.CH, cfg.NCH
    SS, Smax, LASTR = cfg.SS, cfg.Smax, cfg.LASTR
    J, NG = cfg.J, cfg.NG

    # ---- I/O ----
    hfullT = nc.declare_dram_parameter("hfullT", [P, TAB], BF16, isOutput=False)
    hlocT = nc.declare_dram_parameter("hlocT", [P, cfg.NROWS], BF16,
                                      isOutput=False)
    hres2 = nc.declare_dram_parameter("hres2", [cfg.NROWS, P], F32,
                                      isOutput=False)
    constsW = nc.declare_dram_parameter("constsW", [P, 2 * P], BF16,
                                        isOutput=False)
    constsF = nc.declare_dram_parameter("constsF", [P, 5], F32, isOutput=False)
    idxa = nc.declare_dram_parameter("idxa", [P, SS * 8], I16, isOutput=False)
    qmask = nc.declare_dram_parameter("qmask", [P, SS], I8, isOutput=False)
    maskb = nc.declare_dram_parameter("maskb", [P, SS], F32, isOutput=False)
    out = nc.declare_dram_parameter("out", [cfg.NROWS, P], F32, isOutput=True)

    # ---- internal DRAM ----
    xl_tab = nc.dram_tensor("xl_tab", [TABX, P], BF16)
    st_in = nc.dram_tensor("st_in", [P, 2], F32)
    st_out = nc.dram_tensor("st_out", [P, 2], F32, addr_space="Shared")

    with tile.TileContext(nc) as tc:
        import contextlib
        with contextlib.ExitStack() as ctx:
            cst = ctx.enter_context(tc.tile_pool(name="cst", bufs=1))
            ps = ctx.enter_context(tc.tile_pool(name="ps", bufs=4, space="PSUM"))
            ps1 = ctx.enter_context(tc.tile_pool(name="ps1", bufs=1,
                                                 space="PSUM"))

            # ================= constants =================
            csWl = cst.tile([P, P], BF16, tag="csWl")
            nc.sync.dma_start(out=csWl[:], in_=constsW[:, 0:P])
            csWr = cst.tile([P, P], BF16, tag="csWr")
            nc.sync.dma_start(out=csWr[:], in_=constsW[:, P:2 * P])
            csF = cst.tile([P, 5], F32, tag="csF")
            nc.sync.dma_start(out=csF[:], in_=constsF[:])
            ia_sb = cst.tile([P, SS * 8], I16, tag="ia_sb")
            nc.sync.dma_start(out=ia_sb[:], in_=idxa[:])
            qm16 = cst.tile([P, SS], I8, tag="qm16")
            nc.sync.dma_start(out=qm16[:], in_=qmask[:])
            msk_sb = cst.tile([P, SS], F32, tag="msk_sb")
            nc.sync.dma_start(out=msk_sb[:], in_=maskb[:])
            msk16 = cst.tile([P, SS], BF16, tag="msk16")
            nc.scalar.copy(msk16[:], msk_sb[:])

            att_col = csF[:, 0:1]
            gam_col = csF[:, 1:2]
            bet_col = csF[:, 2:3]

            ones_col = cst.tile([P, 1], F32, tag="ones_c")
            nc.gpsimd.memset(ones_col[:], 1.0)
            ident = cst.tile([P, P], F32, tag="ident")
            make_identity(nc, ident[:])
            eps_col = cst.tile([P, 1], F32, tag="epsc")
            nc.gpsimd.memset(eps_col[:], BN_EPS)
            ones_msk = cst.tile([P, 1], F32, tag="ones_m")
            if LASTR < P:
                pidx = cst.tile([P, 1], I32, tag="pidx")
                nc.gpsimd.iota(pidx[:], pattern=[[0, 1]], channel_multiplier=1)
                nc.vector.tensor_scalar(out=ones_msk[:], in0=pidx[:],
                                        scalar1=LASTR, scalar2=None,
                                        op0=mybir.AluOpType.is_lt)
            else:
                nc.gpsimd.memset(ones_msk[:], 1.0)

            # att replicated to all partitions (bf16 row), pre-scaled 0.6
            att_ps = ps.tile([P, P], F32, tag="p1")
            nc.tensor.transpose(att_ps[:], att_col.to_broadcast([P, P]),
                                ident[:])
            att16 = cst.tile([P, P], BF16, tag="att16")
            nc.scalar.activation(att16[:], att_ps[:],
                                 mybir.ActivationFunctionType.Copy,
                                 scale=(1.0 + NEG_SLOPE) / 2.0)
            # bias_l / bias_r replicated to all partitions (f32 rows)
            bl_ps = ps.tile([P, P], F32, tag="p1")
            nc.tensor.transpose(bl_ps[:], csF[:, 3:4].to_broadcast([P, P]),
                                ident[:])
            bl_rep = cst.tile([P, P], F32, tag="blrep")
            nc.scalar.copy(bl_rep[:], bl_ps[:])
            br_ps = ps.tile([P, P], F32, tag="p1")
            nc.tensor.transpose(br_ps[:], csF[:, 4:5].to_broadcast([P, P]),
                                ident[:])
            br_rep = cst.tile([P, P], F32, tag="brrep")
            nc.scalar.copy(br_rep[:], br_ps[:])

            # resident per-window data
            xr16 = cst.tile([P, cfg.NROWS], BF16, tag="xr16")
            xr32 = cst.tile([P, cfg.NROWS], F32, tag="xr32")
            outpre = []
            for w in range(W):
                op_w = cst.tile([P, P], F32, tag=f"op{w}")
                outpre.append(op_w)

            # ================= phase 1: xl table (all nodes) =================
            with tc.tile_pool(name="sb1", bufs=3) as sb1:
                # dedicated all-zero rows at both ends of the table
                ztile = sb1.tile([P, P], BF16, tag="ztile")
                nc.gpsimd.memset(ztile[:], 0.0)
                nc.sync.dma_start(
                    out=xl_tab[0:P, :].rearrange("(p x) f -> p (x f)", p=P),
                    in_=ztile[:])
                nc.sync.dma_start(
                    out=xl_tab[TAB + P:TABX, :]
                        .rearrange("(p x) f -> p (x f)", p=P),
                    in_=ztile[:])

                for c in range(NCH):
                    wd = min(CH, NT - c * CH)
                    c0 = c * CH * P
                    hc = sb1.tile([P, CH * P], BF16, tag="hc")
                    nc.sync.dma_start(out=hc[:, :wd * P],
                                      in_=hfullT[:, c0:c0 + wd * P])
                    xlc = sb1.tile([P, CH * P], BF16, tag="xlc")
                    for j in range(wd):
                        p1 = ps.tile([P, P], F32, tag="p1")
                        nc.tensor.matmul(p1[:], lhsT=hc[:, j * P:(j + 1) * P],
                                         rhs=csWl[:],
                                         start=True, stop=True)
                        nc.vector.tensor_add(xlc[:, j * P:(j + 1) * P],
                                             p1[:], bl_rep[:])
                    # rows at P+c0 stored partition-major: row = P+c0+p*wd+j
                    nc.sync.dma_start(
                        out=xl_tab[P + c0:P + c0 + wd * P, :]
                            .rearrange("(p x) f -> p (x f)", p=P),
                        in_=xlc[:, :wd * P])

                # ---- phase 1b: xr for local (permuted) nodes ----
                hl = sb1.tile([P, cfg.NROWS], BF16, tag="hl")
                nc.sync.dma_start(out=hl[:], in_=hlocT[:])
                for w in range(W):
                    p1 = ps.tile([P, P], F32, tag="p1")
                    nc.tensor.matmul(p1[:], lhsT=hl[:, w * P:(w + 1) * P],
                                     rhs=csWr[:],
                                     start=True, stop=True)
                    nc.vector.tensor_add(xr16[:, w * P:(w + 1) * P],
                                         p1[:], br_rep[:])
                    # upcast of the bf16 value => exact cancellation later
                    nc.vector.tensor_copy(xr32[:, w * P:(w + 1) * P],
                                          xr16[:, w * P:(w + 1) * P])

            tc.strict_bb_all_engine_barrier()

            # ================= phase 2: per-window edge processing ==========
            stats_ps = ps1.tile([P, 2], F32, tag="stats")
            with tc.tile_pool(name="sb2", bufs=2) as sb2:
                for w in range(W):
                    S = cfg.S[w]
                    off = cfg.offs[w]
                    NI = S * P
                    wsl = slice(w * P, (w + 1) * P)

                    G2 = sb2.tile([P, Smax, 2 * P], BF16, tag="G2")
                    if no_gather:
                        nc.gpsimd.memset(G2[:, :S, :], 0.01)
                    else:
                        nc.gpsimd.dma_gather(
                            out_ap=G2[:, :S, :],
                            in_ap=xl_tab[:].rearrange(
                                "(k two) f -> k (two f)", two=2),
                            idxs_ap=ia_sb[:, off * 8:(off + S) * 8],
                            num_idxs=NI, num_idxs_reg=NI, elem_size=2 * P,
                            queue_num=0, single_packet=False)
                    lo = G2[:, :S, 0:P]
                    # odd rows: overwrite lo with hi where qmask
                    nc.vector.copy_predicated(
                        lo, qm16[:, off:off + S][:, :, None]
                            .to_broadcast([P, S, P]),
                        G2[:, :S, P:2 * P])

                    # y = xl[src] + xr[dst]  (dst == own row)
                    Y = sb2.tile([P, Smax, P], BF16, tag="Y")
                    nc.vector.tensor_add(
                        Y[:, :S, :], lo,
                        xr16[:, wsl][:, None, :].to_broadcast([P, S, P]))

                    # z = LeakyReLU(y) = 0.6*y + 0.4*|y|; zz = z*att (inplace)
                    AB = sb2.tile([P, Smax, P], BF16, tag="AB")
                    nc.scalar.activation(
                        AB[:, :S, :], Y[:, :S, :],
                        mybir.ActivationFunctionType.Abs,
                        scale=(1.0 - NEG_SLOPE) / (1.0 + NEG_SLOPE))
                    nc.vector.tensor_add(AB[:, :S, :], AB[:, :S, :],
                                         Y[:, :S, :])
                    nc.vector.tensor_mul(
                        AB[:, :S, :], AB[:, :S, :],
                        att16[:, None, :].to_broadcast([P, S, P]))

                    # scores [p, h, s] = sum_c zz
                    s16 = sb2.tile([P, H, Smax], F32, tag="s16")
                    nc.vector.tensor_reduce(
                        out=s16[:, :, :S].rearrange("p h s -> p s h")
                            [:, :, :, None],
                        in_=AB[:, :S, :].rearrange("p s (h c) -> p s h c",
                                                   c=C),
                        op=mybir.AluOpType.add, axis=mybir.AxisListType.X)
                    # zero pad slots (multiplicative mask)
                    sm = sb2.tile([P, H, Smax], F32, tag="sm")
                    nc.vector.tensor_mul(
                        sm[:, :, :S], s16[:, :, :S],
                        msk_sb[:, off:off + S][:, None, :]
                            .to_broadcast([P, H, S]))
                    # segment-SUM subtraction (reference quirk), exp
                    m = sb2.tile([P, H], F32, tag="m")
                    nc.vector.tensor_reduce(
                        out=m[:, :, None], in_=sm[:, :, :S],
                        op=mybir.AluOpType.add, axis=mybir.AxisListType.X)
                    d = sb2.tile([P, H, Smax], F32, tag="d")
                    nc.vector.tensor_sub(
                        d[:, :, :S], sm[:, :, :S],
                        m[:, :, None].to_broadcast([P, H, S]))
                    e16 = sb2.tile([P, H, Smax], BF16, tag="e16")
                    nc.scalar.activation(e16[:, :, :S], d[:, :, :S],
                                         mybir.ActivationFunctionType.Exp)
                    em = sb2.tile([P, H, Smax], BF16, tag="em")
                    nc.vector.tensor_mul(
                        em[:, :, :S], e16[:, :, :S],
                        msk16[:, off:off + S][:, None, :]
                            .to_broadcast([P, H, S]))
                    den = sb2.tile([P, H], F32, tag="den")
                    nc.vector.tensor_reduce(
                        out=den[:, :, None], in_=em[:, :, :S],
                        op=mybir.AluOpType.add, axis=mybir.AxisListType.X)
                    den2 = sb2.tile([P, H], F32, tag="den2")
                    nc.vector.tensor_scalar(out=den2[:], in0=den[:],
                                            scalar1=1e-16, scalar2=None,
                                            op0=mybir.AluOpType.add)
                    rec = sb2.tile([P, H], F32, tag="rec")
                    nc.vector.reciprocal(rec[:], den2[:])
                    fden = sb2.tile([P, H], F32, tag="fden")
                    nc.vector.tensor_mul(fden[:], den[:], rec[:])

                    # weighted aggregation of y, then /(den+eps) and -xr*f
                    WM = sb2.tile([P, Smax, P], BF16, tag="WM")
                    nc.vector.tensor_mul(
                        WM[:, :S, :].rearrange("p s (h c) -> p s h c", c=C),
                        Y[:, :S, :].rearrange("p s (h c) -> p s h c", c=C),
                        em[:, :, :S].rearrange("p h s -> p s h")
                            [:, :, :, None].to_broadcast([P, S, H, C]))
                    op_w = outpre[w]
                    nc.vector.tensor_reduce(
                        out=op_w[:].rearrange("p (h c) -> p h c", c=C)
                            [:, :, :, None],
                        in_=WM[:, :S, :].rearrange("p s (h c) -> p h c s",
                                                   c=C),
                        op=mybir.AluOpType.add, axis=mybir.AxisListType.X)
                    nc.vector.tensor_mul(
                        op_w[:].rearrange("p (h c) -> p h c", c=C),
                        op_w[:].rearrange("p (h c) -> p h c", c=C),
                        rec[:, :, None].to_broadcast([P, H, C]))
                    xrf = sb2.tile([P, P], F32, tag="xrf")
                    nc.vector.tensor_mul(
                        xrf[:].rearrange("p (h c) -> p h c", c=C),
                        xr32[:, wsl].rearrange("p (h c) -> p h c", c=C),
                        fden[:, :, None].to_broadcast([P, H, C]))
                    nc.vector.tensor_sub(op_w[:], op_w[:], xrf[:])

                    # BN stats accumulation
                    sq = sb2.tile([P, P], F32, tag="sq")
                    nc.vector.tensor_mul(sq[:], op_w[:], op_w[:])
                    stat_ones = ones_msk if w == W - 1 else ones_col
                    nc.tensor.matmul(stats_ps[:, 0:1], lhsT=op_w[:],
                                     rhs=stat_ones[:],
                                     start=(w == 0), stop=(w == W - 1))
                    nc.tensor.matmul(stats_ps[:, 1:2], lhsT=sq[:],
                                     rhs=stat_ones[:],
                                     start=(w == 0), stop=(w == W - 1))

            # ================= phase 3: BN stats AllReduce =================
            with tc.tile_pool(name="sb3", bufs=2) as sb:
                st_sb = sb.tile([P, 2], F32, tag="stsb")
                nc.scalar.copy(st_sb[:], stats_ps[:])
                nc.sync.dma_start(out=st_in[:], in_=st_sb[:])
                tc.strict_bb_all_engine_barrier()
                nc.gpsimd.collective_compute(
                    "AllReduce", mybir.AluOpType.add,
                    replica_groups=[list(range(cfg.n_cores))],
                    ins=[st_in[:]], outs=[st_out[:]])
                tc.strict_bb_all_engine_barrier()
                st_all = sb.tile([P, 2], F32, tag="stall")
                nc.sync.dma_start(out=st_all[:], in_=st_out[:])

                # A = gamma * rsqrt(var+eps); B = beta - A*mu  (y = A*x + B)
                mu_c = sb.tile([P, 1], F32, tag="mu")
                nc.scalar.mul(mu_c[:], st_all[:, 0:1], 1.0 / N)
                ex2 = sb.tile([P, 1], F32, tag="ex2")
                nc.scalar.mul(ex2[:], st_all[:, 1:2], 1.0 / N)
                mu2 = sb.tile([P, 1], F32, tag="mu2")
                nc.scalar.square(mu2[:], mu_c[:])
                var_c = sb.tile([P, 1], F32, tag="var")
                nc.vector.tensor_sub(var_c[:], ex2[:], mu2[:])
                sd = sb.tile([P, 1], F32, tag="sd")
                nc.scalar.activation(sd[:], var_c[:],
                                     mybir.ActivationFunctionType.Sqrt,
                                     bias=eps_col[:])
                rsd = sb.tile([P, 1], F32, tag="rsd")
                nc.vector.reciprocal(rsd[:], sd[:])
                A_c = sb.tile([P, 1], F32, tag="Ac")
                nc.vector.tensor_mul(A_c[:], gam_col, rsd[:])
                Amu = sb.tile([P, 1], F32, tag="Amu")
                nc.vector.tensor_mul(Amu[:], A_c[:], mu_c[:])
                B_c = sb.tile([P, 1], F32, tag="Bc")
                nc.vector.tensor_sub(B_c[:], bet_col, Amu[:])

                A_ps = ps.tile([P, P], F32, tag="p1")
                nc.tensor.transpose(A_ps[:], A_c[:].to_broadcast([P, P]),
                                    ident[:])
                A_rep = cst.tile([P, P], F32, tag="Arep")
                nc.scalar.copy(A_rep[:], A_ps[:])
                B_ps = ps.tile([P, P], F32, tag="p1")
                nc.tensor.transpose(B_ps[:], B_c[:].to_broadcast([P, P]),
                                    ident[:])
                B_rep = cst.tile([P, P], F32, tag="Brep")
                nc.scalar.copy(B_rep[:], B_ps[:])

                # ============ phase 4: BN apply + relu + residual ==========
                for g in range(NG):
                    hres = sb.tile([P, J, P], F32, tag="hres")
                    nc.sync.dma_start(
                        out=hres[:],
                        in_=hres2[g * J * P:(g + 1) * J * P, :]
                            .rearrange("(p j) f -> p j f", p=P))
                    obuf = sb.tile([P, J, P], F32, tag="obuf")
                    for j in range(J):
                        w = g * J + j
                        t1 = sb.tile([P, P], F32, tag="t1")
                        nc.vector.tensor_mul(t1[:], outpre[w][:], A_rep[:])
                        t2 = sb.tile([P, P], F32, tag="t2")
                        nc.vector.tensor_add(t2[:], t1[:], B_rep[:])
                        r = sb.tile([P, P], F32, tag="r")
                        nc.scalar.activation(
                            r[:], t2[:], mybir.ActivationFunctionType.Relu)
                        nc.vector.tensor_add(obuf[:, j, :], r[:],
                                             hres[:, j, :])
                    nc.sync.dma_start(
                        out=out[g * J * P:(g + 1) * J * P, :]
                            .rearrange("(p j) f -> p j f", p=P),
                        in_=obuf[:])
    return nc


def host_prepare(h, edge_index, W_l, W_r, bias_l, bias_r, att,
                 bias_out, gamma, beta, n_cores=8):
    N, D = h.shape
    H, C = att.shape
    h = np.asarray(h, np.float32)
    ei = np.asarray(edge_index)

    loops = np.arange(N, dtype=np.int64)
    src = np.concatenate([ei[0], loops]).astype(np.int64)
    dst = np.concatenate([ei[1], loops]).astype(np.int64)
    deg = np.bincount(dst, minlength=N).astype(np.int64)

    # global degree-desc ordering, striped over cores: rank r -> core r%8
    order = np.argsort(-deg, kind="stable")
    NPC = N // n_cores
    node_of = order.reshape(NPC, n_cores)        # [pos, core]
    degmat = deg[node_of]                        # [pos, core]

    W = math.ceil(NPC / P)
    S = np.zeros(W, np.int64)
    for w in range(W):
        i0, i1 = w * P, min((w + 1) * P, NPC)
        S[w] = max(1, degmat[i0:i1].max())
    cfg = Cfg(N=N, D=D, H=H, n_cores=n_cores, S=S)

    # xl table row permutation (chunked partition-major storage)
    n = np.arange(cfg.TAB, dtype=np.int64)
    cc = n // (cfg.CH * P)
    jj = (n % (cfg.CH * P)) // P
    pp = n % P
    width = np.minimum(cfg.CH, cfg.NT - cc * cfg.CH)
    pi = cc * cfg.CH * P + pp * width + jj
    # node n -> table row P + pi[n]
    rowof = P + pi

    # edges grouped by dst, sorted by table row within each group
    eorder = np.lexsort((rowof[src], dst))
    row_s = rowof[src[eorder]].astype(np.int64)
    starts = np.zeros(N + 1, np.int64)
    starts[1:] = np.cumsum(deg)

    # shared inputs
    hT = np.zeros((P, cfg.TAB), BFNP)
    hT[:, :N] = h.T.astype(BFNP)
    constsW = np.zeros((P, 2 * P), BFNP)
    constsW[:, 0:P] = np.asarray(W_l, np.float32).astype(BFNP)
    constsW[:, P:2 * P] = np.asarray(W_r, np.float32).astype(BFNP)
    constsF = np.zeros((P, 5), np.float32)
    constsF[:, 0] = np.asarray(att, np.float32).reshape(-1)
    constsF[:, 1] = np.asarray(gamma, np.float32)
    constsF[:, 2] = np.asarray(beta, np.float32)
    constsF[:, 3] = np.asarray(bias_l, np.float32)
    constsF[:, 4] = np.asarray(bias_r, np.float32)

    # output/hres row packing: row2(w, p) = (w//J)*J*P + p*J + (w%J)
    wq, wr = np.divmod(np.arange(W), cfg.J)
    r2map = (wq[:, None] * (cfg.J * P) + np.arange(P)[None, :] * cfg.J
             + wr[:, None])                      # [W, P]

    offs = np.asarray(cfg.offs)
    pos = np.arange(NPC)
    w_k = pos // P
    p_k = pos % P
    colstart = offs[w_k]

    in_maps = []
    for k in range(n_cores):
        nodes = node_of[:, k]
        d_k = deg[nodes]
        tot = int(d_k.sum())
        cum = np.zeros(NPC + 1, np.int64)
        cum[1:] = np.cumsum(d_k)
        within = np.arange(tot, dtype=np.int64) - np.repeat(cum[:-1], d_k)
        rows = np.repeat(p_k, d_k)
        cols = np.repeat(colstart, d_k) + within
        eidx = np.repeat(starts[nodes], d_k) + within

        # slot table row values; pads gather the front zero pair
        vals = np.full((P, cfg.SS), 0, np.int64)
        vals[rows, cols] = row_s[eidx]
        va = vals >> 1                                   # pair index
        qm = (vals & 1).astype(np.int8)                  # hi/lo half
        maskba = np.zeros((P, cfg.SS), np.float32)
        maskba[rows, cols] = 1.0

        # dma_gather index layout: flat i = s*128+p; wrapped in 16
        # partitions, replicated down all 128 partitions
        def mk_idx(vmat):
            blocks = []
            for w in range(cfg.W):
                o, s_w = cfg.offs[w], cfg.S[w]
                flat = vmat[:, o:o + s_w].T.reshape(-1)   # [s*128+p]
                blk = flat.reshape(s_w * 8, 16).T         # [16, s*8]
                blocks.append(np.tile(blk, (8, 1)))
            return np.concatenate(blocks, axis=1).astype(np.int16)

        idxa = mk_idx(va)

        hlocT = np.zeros((P, cfg.NROWS), BFNP)
        hlocT[:, :NPC] = h[nodes].T.astype(BFNP)
        hres2 = np.zeros((cfg.NROWS, P), np.float32)
        hres2[r2map[w_k, p_k]] = h[nodes]

        in_maps.append({
            "hfullT": hT, "hlocT": hlocT, "hres2": hres2,
            "constsW": constsW, "constsF": constsF,
            "idxa": idxa, "qmask": qm, "maskb": maskba,
        })

    meta = {"node_of": node_of, "r2map": r2map, "w_k": w_k, "p_k": p_k}
    return cfg, in_maps, meta


def assemble_output(results, cfg, meta, n_cores=8):
    N = cfg.N
    rowsel = meta["r2map"][meta["w_k"], meta["p_k"]]   # [NPC]
    out_full = np.zeros((N, cfg.D), np.float32)
    for k in range(n_cores):
        vals = np.asarray(results[k]["out"], np.float32)[rowsel]
        out_full[meta["node_of"][:, k]] = vals
    return out_full


def kernel(h, edge_index, W_l, W_r, bias_l, bias_r, att,
           bias_out, gamma, beta):
    n_cores = 8
    cfg, in_maps, meta = host_prepare(h, edge_index, W_l, W_r, bias_l,
                                      bias_r, att, bias_out, gamma, beta,
                                      n_cores=n_cores)
    nc = bacc.Bacc(num_swdge_queues=2)
    build_kernel(nc, cfg)
    nc.compile()
    res = run_bass_kernel_spmd(nc, in_maps, core_ids=list(range(n_cores)))
    return assemble_output(res.results, cfg, meta, n_cores=n_cores)


# revision 15
# speedup vs baseline: 1.7223x; 1.6428x over previous
"""GATv2 layer (N=50000, D=128, H=4, E=600000) on 8 trn2 NeuronCores.

Layout: one destination node per SBUF partition row. Nodes are globally
sorted by in-degree, striped across the 8 cores (rank % 8), and packed
into 49 windows of 128 nodes per core; window w pads every node's edge
list to S[w] slots (S[w] = max degree in that window across cores), so
high-degree nodes share windows and padding stays small.

Per window: two dma_gather instructions pull xl[src] rows (bf16, 256B
elements) for all 128*S[w] edge slots — the xl table is split at row
32768 because dma_gather indices are signed int16; slots whose row
lives in the other half gather a dedicated all-zero row, so the halves
just add (no select). xr[dst] is the node's own row broadcast along the
slot axis (no second gather, no selection matmuls). Segment "softmax"
and the alpha-weighted aggregation are row-local vector reduces.

NOTE: this environment's jax.ops.segment_max computes a segment SUM;
the reference subtracts that (not the max) before exp and divides by
(den + 1e-16). We reproduce both quirks exactly — they change the
output materially (some heads collapse to ~0 when den << 1e-16).

xl table rows are stored chunk-permuted (pi) so the phase-1 table write
is one contiguous 4KB-per-partition DMA per 16-tile chunk; the host
remaps gather indices accordingly and un-permutes the output.
"""

import math
import numpy as np
import ml_dtypes

import concourse.bass as bass
import concourse.bacc as bacc
import concourse.mybir as mybir
import concourse.tile as tile
from concourse.masks import make_identity
from concourse.bass_utils import run_bass_kernel_spmd

P = 128
F32 = mybir.dt.float32
BF16 = mybir.dt.bfloat16
I32 = mybir.dt.int32
I16 = mybir.dt.int16
I8 = mybir.dt.int8
BFNP = ml_dtypes.bfloat16

NEG_SLOPE = 0.2
BN_EPS = 1e-5
HALF = 32768          # dma_gather signed-int16 index limit


class Cfg:
    def __init__(self, N, D, H, n_cores, S):
        assert D == P
        self.N, self.D, self.H = N, D, H
        self.C = D // H
        self.n_cores = n_cores
        self.NPC = N // n_cores              # nodes per core
        self.W = math.ceil(self.NPC / P)     # windows per core
        self.NROWS = self.W * P
        self.LASTR = self.NPC - P * (self.W - 1)
        self.NT = math.ceil(N / P)           # xl table tiles
        self.TAB = self.NT * P
        # [zeros 128] [table TAB] [zeros 128]
        self.TABX = self.TAB + 2 * P
        self.BZERO = self.TAB + P - HALF     # back zero row, rel to HALF
        self.CH = 16                         # tiles per xl-table write chunk
        self.NCH = math.ceil(self.NT / self.CH)
        self.S = [int(s) for s in S]         # slots per window
        offs = np.concatenate([[0], np.cumsum(self.S)])
        self.offs = [int(o) for o in offs]
        self.SS = int(offs[-1])
        self.Smax = int(max(self.S))
        self.J = 7                           # windows per output write group
        self.NG = self.W // self.J
        assert self.W == self.J * self.NG


def build_kernel(nc: bass.Bass, cfg: Cfg, no_gather=False, single_q=False):
    N, H, C, W = cfg.N, cfg.H, cfg.C, cfg.W
    NT, TAB, TABX, CH, NCH = cfg.NT, cfg.TAB, cfg.TABX, cfg.CH, cfg.NCH
    SS, Smax, LASTR = cfg.SS, cfg.Smax, cfg.LASTR
    J, NG = cfg.J, cfg.NG

    # ---- I/O ----
    hfullT = nc.declare_dram_parameter("hfullT", [P, TAB], BF16, isOutput=False)
    hlocT = nc.declare_dram_parameter("hlocT", [P, cfg.NROWS], BF16,
                                      isOutput=False)
    hres2 = nc.declare_dram_parameter("hres2", [cfg.NROWS, P], F32,
                                      isOutput=False)
    constsW = nc.declare_dram_parameter("constsW", [P, 2 * P], BF16,
                                        isOutput=False)
    constsF = nc.declare_dram_parameter("constsF", [P, 5], F32, isOutput=False)
    idxa = nc.declare_dram_parameter("idxa", [P, SS * 8], I16, isOutput=False)
    qmask = nc.declare_dram_parameter("qmask", [P, SS], I8, isOutput=False)
    maskb = nc.declare_dram_parameter("maskb", [P, SS], F32, isOutput=False)
    out = nc.declare_dram_parameter("out", [cfg.NROWS, P], F32, isOutput=True)

    # ---- internal DRAM ----
    xl_tab = nc.dram_tensor("xl_tab", [TABX, P], BF16)
    st_in = nc.dram_tensor("st_in", [P, 2], F32)
    st_out = nc.dram_tensor("st_out", [P, 2], F32, addr_space="Shared")

    with tile.TileContext(nc) as tc:
        import contextlib
        with contextlib.ExitStack() as ctx:
            cst = ctx.enter_context(tc.tile_pool(name="cst", bufs=1))
            ps = ctx.enter_context(tc.tile_pool(name="ps", bufs=4, space="PSUM"))
            ps1 = ctx.enter_context(tc.tile_pool(name="ps1", bufs=1,
                                                 space="PSUM"))

            # ================= constants =================
            csWl = cst.tile([P, P], BF16, tag="csWl")
            nc.sync.dma_start(out=csWl[:], in_=constsW[:, 0:P])
            csWr = cst.tile([P, P], BF16, tag="csWr")
            nc.sync.dma_start(out=csWr[:], in_=constsW[:, P:2 * P])
            csF = cst.tile([P, 5], F32, tag="csF")
            nc.sync.dma_start(out=csF[:], in_=constsF[:])
            ia_sb = cst.tile([P, SS * 8], I16, tag="ia_sb")
            nc.sync.dma_start(out=ia_sb[:], in_=idxa[:])
            qm16 = cst.tile([P, SS], I8, tag="qm16")
            nc.sync.dma_start(out=qm16[:], in_=qmask[:])
            msk_sb = cst.tile([P, SS], F32, tag="msk_sb")
            nc.sync.dma_start(out=msk_sb[:], in_=maskb[:])
            msk16 = cst.tile([P, SS], BF16, tag="msk16")
            nc.scalar.copy(msk16[:], msk_sb[:])

            att_col = csF[:, 0:1]
            gam_col = csF[:, 1:2]
            bet_col = csF[:, 2:3]

            ones_col = cst.tile([P, 1], F32, tag="ones_c")
            nc.gpsimd.memset(ones_col[:], 1.0)
            ident = cst.tile([P, P], F32, tag="ident")
            make_identity(nc, ident[:])
            eps_col = cst.tile([P, 1], F32, tag="epsc")
            nc.gpsimd.memset(eps_col[:], BN_EPS)
            ones_msk = cst.tile([P, 1], F32, tag="ones_m")
            if LASTR < P:
                pidx = cst.tile([P, 1], I32, tag="pidx")
                nc.gpsimd.iota(pidx[:], pattern=[[0, 1]], channel_multiplier=1)
                nc.vector.tensor_scalar(out=ones_msk[:], in0=pidx[:],
                                        scalar1=LASTR, scalar2=None,
                                        op0=mybir.AluOpType.is_lt)
            else:
                nc.gpsimd.memset(ones_msk[:], 1.0)

            # att replicated to all partitions (bf16 row), pre-scaled 0.6
            att_ps = ps.tile([P, P], F32, tag="p1")
            nc.tensor.transpose(att_ps[:], att_col.to_broadcast([P, P]),
                                ident[:])
            att16 = cst.tile([P, P], BF16, tag="att16")
            nc.scalar.activation(att16[:], att_ps[:],
                                 mybir.ActivationFunctionType.Copy,
                                 scale=(1.0 + NEG_SLOPE) / 2.0)
            # bias_l / bias_r replicated to all partitions (f32 rows)
            bl_ps = ps.tile([P, P], F32, tag="p1")
            nc.tensor.transpose(bl_ps[:], csF[:, 3:4].to_broadcast([P, P]),
                                ident[:])
            bl_rep = cst.tile([P, P], F32, tag="blrep")
            nc.scalar.copy(bl_rep[:], bl_ps[:])
            br_ps = ps.tile([P, P], F32, tag="p1")
            nc.tensor.transpose(br_ps[:], csF[:, 4:5].to_broadcast([P, P]),
                                ident[:])
            br_rep = cst.tile([P, P], F32, tag="brrep")
            nc.scalar.copy(br_rep[:], br_ps[:])

            # resident per-window data
            xr16 = cst.tile([P, cfg.NROWS], BF16, tag="xr16")
            xr32 = cst.tile([P, cfg.NROWS], F32, tag="xr32")
            outpre = []
            for w in range(W):
                op_w = cst.tile([P, P], F32, tag=f"op{w}")
                outpre.append(op_w)

            # ================= phase 1: xl table (all nodes) =================
            with tc.tile_pool(name="sb1", bufs=3) as sb1:
                # dedicated all-zero rows at both ends of the table
                ztile = sb1.tile([P, P], BF16, tag="ztile")
                nc.gpsimd.memset(ztile[:], 0.0)
                nc.sync.dma_start(
                    out=xl_tab[0:P, :].rearrange("(p x) f -> p (x f)", p=P),
                    in_=ztile[:])
                nc.sync.dma_start(
                    out=xl_tab[TAB + P:TABX, :]
                        .rearrange("(p x) f -> p (x f)", p=P),
                    in_=ztile[:])

                for c in range(NCH):
                    wd = min(CH, NT - c * CH)
                    c0 = c * CH * P
                    hc = sb1.tile([P, CH * P], BF16, tag="hc")
                    nc.sync.dma_start(out=hc[:, :wd * P],
                                      in_=hfullT[:, c0:c0 + wd * P])
                    xlc = sb1.tile([P, CH * P], BF16, tag="xlc")
                    for j in range(wd):
                        p1 = ps.tile([P, P], F32, tag="p1")
                        nc.tensor.matmul(p1[:], lhsT=hc[:, j * P:(j + 1) * P],
                                         rhs=csWl[:],
                                         start=True, stop=True)
                        nc.vector.tensor_add(xlc[:, j * P:(j + 1) * P],
                                             p1[:], bl_rep[:])
                    # rows at P+c0 stored partition-major: row = P+c0+p*wd+j
                    nc.sync.dma_start(
                        out=xl_tab[P + c0:P + c0 + wd * P, :]
                            .rearrange("(p x) f -> p (x f)", p=P),
                        in_=xlc[:, :wd * P])

                # ---- phase 1b: xr for local (permuted) nodes ----
                hl = sb1.tile([P, cfg.NROWS], BF16, tag="hl")
                nc.sync.dma_start(out=hl[:], in_=hlocT[:])
                for w in range(W):
                    p1 = ps.tile([P, P], F32, tag="p1")
                    nc.tensor.matmul(p1[:], lhsT=hl[:, w * P:(w + 1) * P],
                                     rhs=csWr[:],
                                     start=True, stop=True)
                    nc.vector.tensor_add(xr16[:, w * P:(w + 1) * P],
                                         p1[:], br_rep[:])
                    # upcast of the bf16 value => exact cancellation later
                    nc.vector.tensor_copy(xr32[:, w * P:(w + 1) * P],
                                          xr16[:, w * P:(w + 1) * P])

            tc.strict_bb_all_engine_barrier()

            # ================= phase 2: per-window edge processing ==========
            stats_ps = ps1.tile([P, 2], F32, tag="stats")
            with tc.tile_pool(name="sb2", bufs=2) as sb2, \
                 tc.tile_pool(name="sbg", bufs=3) as sbg:
                for w in range(W):
                    S = cfg.S[w]
                    off = cfg.offs[w]
                    NI = S * P
                    wsl = slice(w * P, (w + 1) * P)

                    G2 = sbg.tile([P, Smax, 2 * P], BF16, tag="G2")
                    if no_gather:
                        nc.gpsimd.memset(G2[:, :S, :], 0.01)
                    else:
                        nc.gpsimd.dma_gather(
                            out_ap=G2[:, :S, :],
                            in_ap=xl_tab[:].rearrange(
                                "(k two) f -> k (two f)", two=2),
                            idxs_ap=ia_sb[:, off * 8:(off + S) * 8],
                            num_idxs=NI, num_idxs_reg=NI, elem_size=2 * P,
                            queue_num=0, single_packet=False)
                    lo = G2[:, :S, 0:P]
                    # odd rows: overwrite lo with hi where qmask
                    nc.vector.copy_predicated(
                        lo, qm16[:, off:off + S][:, :, None]
                            .to_broadcast([P, S, P]),
                        G2[:, :S, P:2 * P])

                    # y = xl[src] + xr[dst]  (dst == own row)
                    Y = sb2.tile([P, Smax, P], BF16, tag="Y")
                    nc.vector.tensor_add(
                        Y[:, :S, :], lo,
                        xr16[:, wsl][:, None, :].to_broadcast([P, S, P]))

                    # z = LeakyReLU(y) = 0.6*y + 0.4*|y|; zz = z*att (inplace)
                    AB = sb2.tile([P, Smax, P], BF16, tag="AB")
                    nc.scalar.activation(
                        AB[:, :S, :], Y[:, :S, :],
                        mybir.ActivationFunctionType.Abs,
                        scale=(1.0 - NEG_SLOPE) / (1.0 + NEG_SLOPE))
                    nc.vector.tensor_add(AB[:, :S, :], AB[:, :S, :],
                                         Y[:, :S, :])
                    nc.vector.tensor_mul(
                        AB[:, :S, :], AB[:, :S, :],
                        att16[:, None, :].to_broadcast([P, S, P]))

                    # scores [p, h, s] = sum_c zz
                    s16 = sb2.tile([P, H, Smax], F32, tag="s16")
                    nc.vector.tensor_reduce(
                        out=s16[:, :, :S].rearrange("p h s -> p s h")
                            [:, :, :, None],
                        in_=AB[:, :S, :].rearrange("p s (h c) -> p s h c",
                                                   c=C),
                        op=mybir.AluOpType.add, axis=mybir.AxisListType.X)
                    # zero pad slots (multiplicative mask)
                    sm = sb2.tile([P, H, Smax], F32, tag="sm")
                    nc.vector.tensor_mul(
                        sm[:, :, :S], s16[:, :, :S],
                        msk_sb[:, off:off + S][:, None, :]
                            .to_broadcast([P, H, S]))
                    # segment-SUM subtraction (reference quirk), exp
                    m = sb2.tile([P, H], F32, tag="m")
                    nc.vector.tensor_reduce(
                        out=m[:, :, None], in_=sm[:, :, :S],
                        op=mybir.AluOpType.add, axis=mybir.AxisListType.X)
                    d = sb2.tile([P, H, Smax], F32, tag="d")
                    nc.vector.tensor_sub(
                        d[:, :, :S], sm[:, :, :S],
                        m[:, :, None].to_broadcast([P, H, S]))
                    e16 = sb2.tile([P, H, Smax], BF16, tag="e16")
                    nc.scalar.activation(e16[:, :, :S], d[:, :, :S],
                                         mybir.ActivationFunctionType.Exp)
                    em = sb2.tile([P, H, Smax], BF16, tag="em")
                    nc.vector.tensor_mul(
                        em[:, :, :S], e16[:, :, :S],
                        msk16[:, off:off + S][:, None, :]
                            .to_broadcast([P, H, S]))
                    den = sb2.tile([P, H], F32, tag="den")
                    nc.vector.tensor_reduce(
                        out=den[:, :, None], in_=em[:, :, :S],
                        op=mybir.AluOpType.add, axis=mybir.AxisListType.X)
                    den2 = sb2.tile([P, H], F32, tag="den2")
                    nc.vector.tensor_scalar(out=den2[:], in0=den[:],
                                            scalar1=1e-16, scalar2=None,
                                            op0=mybir.AluOpType.add)
                    rec = sb2.tile([P, H], F32, tag="rec")
                    nc.vector.reciprocal(rec[:], den2[:])
                    fden = sb2.tile([P, H], F32, tag="fden")
                    nc.vector.tensor_mul(fden[:], den[:], rec[:])

                    # weighted aggregation of y, then /(den+eps) and -xr*f
                    WM = sb2.tile([P, Smax, P], BF16, tag="WM")
                    nc.vector.tensor_mul(
                        WM[:, :S, :].rearrange("p s (h c) -> p s h c", c=C),
                        Y[:, :S, :].rearrange("p s (h c) -> p s h c", c=C),
                        em[:, :, :S].rearrange("p h s -> p s h")
                            [:, :, :, None].to_broadcast([P, S, H, C]))
                    op_w = outpre[w]
                    nc.vector.tensor_reduce(
                        out=op_w[:].rearrange("p (h c) -> p h c", c=C)
                            [:, :, :, None],
                        in_=WM[:, :S, :].rearrange("p s (h c) -> p h c s",
                                                   c=C),
                        op=mybir.AluOpType.add, axis=mybir.AxisListType.X)
                    nc.vector.tensor_mul(
                        op_w[:].rearrange("p (h c) -> p h c", c=C),
                        op_w[:].rearrange("p (h c) -> p h c", c=C),
                        rec[:, :, None].to_broadcast([P, H, C]))
                    xrf = sb2.tile([P, P], F32, tag="xrf")
                    nc.vector.tensor_mul(
                        xrf[:].rearrange("p (h c) -> p h c", c=C),
                        xr32[:, wsl].rearrange("p (h c) -> p h c", c=C),
                        fden[:, :, None].to_broadcast([P, H, C]))
                    nc.vector.tensor_sub(op_w[:], op_w[:], xrf[:])

                    # BN stats accumulation
                    sq = sb2.tile([P, P], F32, tag="sq")
                    nc.vector.tensor_mul(sq[:], op_w[:], op_w[:])
                    stat_ones = ones_msk if w == W - 1 else ones_col
                    nc.tensor.matmul(stats_ps[:, 0:1], lhsT=op_w[:],
                                     rhs=stat_ones[:],
                                     start=(w == 0), stop=(w == W - 1))
                    nc.tensor.matmul(stats_ps[:, 1:2], lhsT=sq[:],
                                     rhs=stat_ones[:],
                                     start=(w == 0), stop=(w == W - 1))

            # ================= phase 3: BN stats AllReduce =================
            with tc.tile_pool(name="sb3", bufs=2) as sb:
                st_sb = sb.tile([P, 2], F32, tag="stsb")
                nc.scalar.copy(st_sb[:], stats_ps[:])
                nc.sync.dma_start(out=st_in[:], in_=st_sb[:])
                tc.strict_bb_all_engine_barrier()
                nc.gpsimd.collective_compute(
                    "AllReduce", mybir.AluOpType.add,
                    replica_groups=[list(range(cfg.n_cores))],
                    ins=[st_in[:]], outs=[st_out[:]])
                tc.strict_bb_all_engine_barrier()
                st_all = sb.tile([P, 2], F32, tag="stall")
                nc.sync.dma_start(out=st_all[:], in_=st_out[:])

                # A = gamma * rsqrt(var+eps); B = beta - A*mu  (y = A*x + B)
                mu_c = sb.tile([P, 1], F32, tag="mu")
                nc.scalar.mul(mu_c[:], st_all[:, 0:1], 1.0 / N)
                ex2 = sb.tile([P, 1], F32, tag="ex2")
                nc.scalar.mul(ex2[:], st_all[:, 1:2], 1.0 / N)
                mu2 = sb.tile([P, 1], F32, tag="mu2")
                nc.scalar.square(mu2[:], mu_c[:])
                var_c = sb.tile([P, 1], F32, tag="var")
                nc.vector.tensor_sub(var_c[:], ex2[:], mu2[:])
                sd = sb.tile([P, 1], F32, tag="sd")
                nc.scalar.activation(sd[:], var_c[:],
                                     mybir.ActivationFunctionType.Sqrt,
                                     bias=eps_col[:])
                rsd = sb.tile([P, 1], F32, tag="rsd")
                nc.vector.reciprocal(rsd[:], sd[:])
                A_c = sb.tile([P, 1], F32, tag="Ac")
                nc.vector.tensor_mul(A_c[:], gam_col, rsd[:])
                Amu = sb.tile([P, 1], F32, tag="Amu")
                nc.vector.tensor_mul(Amu[:], A_c[:], mu_c[:])
                B_c = sb.tile([P, 1], F32, tag="Bc")
                nc.vector.tensor_sub(B_c[:], bet_col, Amu[:])

                A_ps = ps.tile([P, P], F32, tag="p1")
                nc.tensor.transpose(A_ps[:], A_c[:].to_broadcast([P, P]),
                                    ident[:])
                A_rep = cst.tile([P, P], F32, tag="Arep")
                nc.scalar.copy(A_rep[:], A_ps[:])
                B_ps = ps.tile([P, P], F32, tag="p1")
                nc.tensor.transpose(B_ps[:], B_c[:].to_broadcast([P, P]),
                                    ident[:])
                B_rep = cst.tile([P, P], F32, tag="Brep")
                nc.scalar.copy(B_rep[:], B_ps[:])

                # ============ phase 4: BN apply + relu + residual ==========
                for g in range(NG):
                    hres = sb.tile([P, J, P], F32, tag="hres")
                    nc.sync.dma_start(
                        out=hres[:],
                        in_=hres2[g * J * P:(g + 1) * J * P, :]
                            .rearrange("(p j) f -> p j f", p=P))
                    obuf = sb.tile([P, J, P], F32, tag="obuf")
                    for j in range(J):
                        w = g * J + j
                        t1 = sb.tile([P, P], F32, tag="t1")
                        nc.vector.tensor_mul(t1[:], outpre[w][:], A_rep[:])
                        t2 = sb.tile([P, P], F32, tag="t2")
                        nc.vector.tensor_add(t2[:], t1[:], B_rep[:])
                        r = sb.tile([P, P], F32, tag="r")
                        nc.scalar.activation(
                            r[:], t2[:], mybir.ActivationFunctionType.Relu)
                        nc.vector.tensor_add(obuf[:, j, :], r[:],
                                             hres[:, j, :])
                    nc.sync.dma_start(
                        out=out[g * J * P:(g + 1) * J * P, :]
                            .rearrange("(p j) f -> p j f", p=P),
                        in_=obuf[:])
    return nc


def host_prepare(h, edge_index, W_l, W_r, bias_l, bias_r, att,
                 bias_out, gamma, beta, n_cores=8):
    N, D = h.shape
    H, C = att.shape
    h = np.asarray(h, np.float32)
    ei = np.asarray(edge_index)

    loops = np.arange(N, dtype=np.int64)
    src = np.concatenate([ei[0], loops]).astype(np.int64)
    dst = np.concatenate([ei[1], loops]).astype(np.int64)
    deg = np.bincount(dst, minlength=N).astype(np.int64)

    # global degree-desc ordering, striped over cores: rank r -> core r%8
    order = np.argsort(-deg, kind="stable")
    NPC = N // n_cores
    node_of = order.reshape(NPC, n_cores)        # [pos, core]
    degmat = deg[node_of]                        # [pos, core]

    W = math.ceil(NPC / P)
    S = np.zeros(W, np.int64)
    for w in range(W):
        i0, i1 = w * P, min((w + 1) * P, NPC)
        S[w] = max(1, degmat[i0:i1].max())
    cfg = Cfg(N=N, D=D, H=H, n_cores=n_cores, S=S)

    # xl table row permutation (chunked partition-major storage)
    n = np.arange(cfg.TAB, dtype=np.int64)
    cc = n // (cfg.CH * P)
    jj = (n % (cfg.CH * P)) // P
    pp = n % P
    width = np.minimum(cfg.CH, cfg.NT - cc * cfg.CH)
    pi = cc * cfg.CH * P + pp * width + jj
    # node n -> table row P + pi[n]
    rowof = P + pi

    # edges grouped by dst, sorted by table row within each group
    eorder = np.lexsort((rowof[src], dst))
    row_s = rowof[src[eorder]].astype(np.int64)
    starts = np.zeros(N + 1, np.int64)
    starts[1:] = np.cumsum(deg)

    # shared inputs
    hT = np.zeros((P, cfg.TAB), BFNP)
    hT[:, :N] = h.T.astype(BFNP)
    constsW = np.zeros((P, 2 * P), BFNP)
    constsW[:, 0:P] = np.asarray(W_l, np.float32).astype(BFNP)
    constsW[:, P:2 * P] = np.asarray(W_r, np.float32).astype(BFNP)
    constsF = np.zeros((P, 5), np.float32)
    constsF[:, 0] = np.asarray(att, np.float32).reshape(-1)
    constsF[:, 1] = np.asarray(gamma, np.float32)
    constsF[:, 2] = np.asarray(beta, np.float32)
    constsF[:, 3] = np.asarray(bias_l, np.float32)
    constsF[:, 4] = np.asarray(bias_r, np.float32)

    # output/hres row packing: row2(w, p) = (w//J)*J*P + p*J + (w%J)
    wq, wr = np.divmod(np.arange(W), cfg.J)
    r2map = (wq[:, None] * (cfg.J * P) + np.arange(P)[None, :] * cfg.J
             + wr[:, None])                      # [W, P]

    offs = np.asarray(cfg.offs)
    pos = np.arange(NPC)
    w_k = pos // P
    p_k = pos % P
    colstart = offs[w_k]

    in_maps = []
    for k in range(n_cores):
        nodes = node_of[:, k]
        d_k = deg[nodes]
        tot = int(d_k.sum())
        cum = np.zeros(NPC + 1, np.int64)
        cum[1:] = np.cumsum(d_k)
        within = np.arange(tot, dtype=np.int64) - np.repeat(cum[:-1], d_k)
        rows = np.repeat(p_k, d_k)
        cols = np.repeat(colstart, d_k) + within
        eidx = np.repeat(starts[nodes], d_k) + within

        # slot table row values; pads gather the front zero pair
        vals = np.full((P, cfg.SS), 0, np.int64)
        vals[rows, cols] = row_s[eidx]
        va = vals >> 1                                   # pair index
        qm = (vals & 1).astype(np.int8)                  # hi/lo half
        maskba = np.zeros((P, cfg.SS), np.float32)
        maskba[rows, cols] = 1.0

        # dma_gather index layout: flat i = s*128+p; wrapped in 16
        # partitions, replicated down all 128 partitions
        def mk_idx(vmat):
            blocks = []
            for w in range(cfg.W):
                o, s_w = cfg.offs[w], cfg.S[w]
                flat = vmat[:, o:o + s_w].T.reshape(-1)   # [s*128+p]
                blk = flat.reshape(s_w * 8, 16).T         # [16, s*8]
                blocks.append(np.tile(blk, (8, 1)))
            return np.concatenate(blocks, axis=1).astype(np.int16)

        idxa = mk_idx(va)

        hlocT = np.zeros((P, cfg.NROWS), BFNP)
        hlocT[:, :NPC] = h[nodes].T.astype(BFNP)
        hres2 = np.zeros((cfg.NROWS, P), np.float32)
        hres2[r2map[w_k, p_k]] = h[nodes]

        in_maps.append({
            "hfullT": hT, "hlocT": hlocT, "hres2": hres2,
            "constsW": constsW, "constsF": constsF,
            "idxa": idxa, "qmask": qm, "maskb": maskba,
        })

    meta = {"node_of": node_of, "r2map": r2map, "w_k": w_k, "p_k": p_k}
    return cfg, in_maps, meta


def assemble_output(results, cfg, meta, n_cores=8):
    N = cfg.N
    rowsel = meta["r2map"][meta["w_k"], meta["p_k"]]   # [NPC]
    out_full = np.zeros((N, cfg.D), np.float32)
    for k in range(n_cores):
        vals = np.asarray(results[k]["out"], np.float32)[rowsel]
        out_full[meta["node_of"][:, k]] = vals
    return out_full


def kernel(h, edge_index, W_l, W_r, bias_l, bias_r, att,
           bias_out, gamma, beta):
    n_cores = 8
    cfg, in_maps, meta = host_prepare(h, edge_index, W_l, W_r, bias_l,
                                      bias_r, att, bias_out, gamma, beta,
                                      n_cores=n_cores)
    nc = bacc.Bacc(num_swdge_queues=2)
    build_kernel(nc, cfg)
    nc.compile()
    res = run_bass_kernel_spmd(nc, in_maps, core_ids=list(range(n_cores)))
    return assemble_output(res.results, cfg, meta, n_cores=n_cores)
